# revision 7
# baseline (speedup 1.0000x reference)
"""GQA kernel for Trainium2, sharded over 8 NeuronCores.

Problem: x[2,2048,2048] -> GQA(HQ=16 q-heads, HKV=4 kv-heads, D=128) -> out[2,2048,2048]
Sharding: core c = b*4 + h handles batch b and kv-head group h (4 q-heads).
Wq/Wk/Wv column-sharded per head group, Wo row-sharded; partial outputs
summed on host per batch.

v3 schedule (per core, bf16 matmul operands, fp32 PSUM):
  warmup: ~44 dummy matmuls during the input-DMA wait so the PE HAM
          clock gate reaches 8/8 before real work lands.
  phase 1: 24 accumulation groups (4 nb-chunks x [4q,k,v]) in waves of
           8 concurrent PSUM banks, e-tile-outer so matmuls unlock as
           each e-slice's DMA lands. Wave copies alternate DVE/ACT.
           V tiles transposed into va via the DMA XBAR transpose.
  phase 2: one flat software-pipelined stream over all 128 (block, j)
           steps: scores^T (2x512 MMs) -> exp (ACT, 1024-wide) -> 8 AV
           matmuls lag-1 behind (j-outer, 8 concurrent sub-bank
           accumulation regions packed 3/3/2 into 3 PSUM banks; V
           carries [v|ones|0 0 0] so every accumulated byte is
           matmul-written). Pipelining continues across block
           boundaries so ACT never idles. Normalize: bulk PSUM drain +
           per-isub recip/mul; attnT built with DMA XBAR transposes.
  phase 3: two passes (nb 0-1 using ib0 attnT first, then nb 2-3) so
           the last block's transposes overlap pass A; bf16 store.
"""

import math

import numpy as np

B = 2
N = 2048
E = 2048
HQ = 16
G = 4
HKV = 4
D = 128
FQ = G * D  # 512 q-features per group
P = 128
NB = N // 512  # 4 chunks of 512
ET = E // P  # 16 contraction tiles
JT = N // P  # 16 key tiles
IB2 = N // 1024  # 2 query blocks of 1024
SCALE = 1.0 / math.sqrt(D)

RW = 132  # AV region width: 128 v-dims + denom + 3 zero pad (all written)

USE_DMA_TRANSPOSE = False

_CACHE: dict = {}


class _nullpool:
    def __enter__(self):
        return None

    def __exit__(self, *a):
        return False


def _build_program():
    import concourse.bacc as bacc
    import concourse.tile as tile
    from concourse import mybir
    from concourse.masks import make_identity

    f32 = mybir.dt.float32
    bf16 = mybir.dt.bfloat16
    nc = bacc.Bacc("TRN2", target_bir_lowering=False)

    xT_d = nc.dram_tensor("xT", [ET, P, N], bf16, kind="ExternalInput")
    wqT_d = nc.dram_tensor("wqT", [P, ET, FQ], bf16, kind="ExternalInput")
    wkT_d = nc.dram_tensor("wkT", [P, ET, D], bf16, kind="ExternalInput")
    wvT_d = nc.dram_tensor("wvT", [P, ET, D], bf16, kind="ExternalInput")
    woT_d = nc.dram_tensor("woT", [P, G, N], bf16, kind="ExternalInput")
    outT_d = nc.dram_tensor("outT", [ET, P, N], bf16, kind="ExternalOutput")

    with tile.TileContext(nc) as tc:
        with tc.tile_pool(name="persist", bufs=1) as persist, \
             tc.tile_pool(name="w1", bufs=1) as w1, \
             tc.tile_pool(name="et", bufs=3) as etp, \
             tc.tile_pool(name="tp", bufs=2) as tpool, \
             tc.tile_pool(name="small", bufs=4) as small, \
             tc.tile_pool(name="op", bufs=2) as op:
            qT = [persist.tile([P, N], bf16, name=f"qT{f}", tag=f"qT{f}")
                  for f in range(G)]
            kT = persist.tile([P, N], bf16, tag="kT")
            vTs = persist.tile([P, N], bf16, tag="vTs")
            va = persist.tile([P, JT, RW], bf16, tag="va")
            attnT = [persist.tile([P, N], bf16, name=f"attnT{g}", tag=f"attnT{g}")
                     for g in range(G)]
            wo_sb = persist.tile([P, G, N], bf16, tag="wo_sb")
            scratch = persist.tile([P, 192], bf16, tag="scratch")
            if not USE_DMA_TRANSPOSE:
                ident = persist.tile([P, P], bf16, tag="ident")
                make_identity(nc, ident)

            nc.vector.memset(scratch[:], 0.0)
            nc.vector.memset(va[:, :, P:P + 1], 1.0)
            nc.vector.memset(va[:, :, P + 1:RW], 0.0)

            # ---------------- phase 1: projections ----------------
            with tc.tile_pool(name="pp", bufs=1, space="PSUM") as pp:
                wq_sb = w1.tile([P, ET, FQ], bf16, tag="wq_sb")
                wk_sb = w1.tile([P, ET, D], bf16, tag="wk_sb")
                wv_sb = w1.tile([P, ET, D], bf16, tag="wv_sb")
                xts = []
                for e in range(ET):
                    nc.sync.dma_start(out=wq_sb[:, e, :], in_=wqT_d[:, e, :])
                    nc.sync.dma_start(out=wk_sb[:, e, :], in_=wkT_d[:, e, :])
                    nc.sync.dma_start(out=wv_sb[:, e, :], in_=wvT_d[:, e, :])
                    xt = w1.tile([P, N], bf16, name=f"xt{e}", tag=f"xt{e}")
                    nc.sync.dma_start(out=xt[:], in_=xT_d[e])
                    xts.append(xt)
                nc.sync.dma_start(out=wo_sb[:], in_=woT_d[:])

                # PE warmup against the HAM clock gate while DMA streams in
                wt0 = pp.tile([P, 512], f32, name="slot0", tag="slot0")
                for _ in range(44):
                    nc.tensor.matmul(
                        wt0[:, 0:64], scratch[:, 0:P], scratch[:, P:192],
                        start=True, stop=True,
                    )

                def w_slice(t, e):
                    if t < G:
                        return wq_sb[:, e, t * P:(t + 1) * P]
                    if t == G:
                        return wk_sb[:, e, :]
                    return wv_sb[:, e, :]

                waves = [
                    [(0, 0), (0, 1), (0, 2), (0, 3), (0, 4), (0, 5), (1, 0), (1, 1)],
                    [(1, 2), (1, 3), (1, 4), (1, 5), (2, 0), (2, 1), (2, 2), (2, 3)],
                    [(3, 4), (3, 5), (2, 4), (2, 5), (3, 0), (3, 1), (3, 2), (3, 3)],
                ]
                for wave in waves:
                    slots = [pp.tile([P, 512], f32, name=f"slot{i}",
                                     tag=f"slot{i}") for i in range(8)]
                    for e in range(ET):
                        for i, (nb, t) in enumerate(wave):
                            nc.tensor.matmul(
                                slots[i][:],
                                w_slice(t, e),
                                xts[e][:, nb * 512:(nb + 1) * 512],
                                start=(e == 0),
                                stop=(e == ET - 1),
                            )
                    for i, (nb, t) in enumerate(wave):
                        sl = slice(nb * 512, (nb + 1) * 512)
                        eng = nc.vector.tensor_copy if i % 2 == 0 else nc.scalar.copy
                        if t < G:
                            eng(qT[t][:, sl], slots[i][:])
                        elif t == G:
                            eng(kT[:, sl], slots[i][:])
                        else:
                            eng(vTs[:, sl], slots[i][:])
                            if USE_DMA_TRANSPOSE:
                                for j in range(nb * 4, nb * 4 + 4):
                                    nc.sync.dma_start_transpose(
                                        va[:, j, 0:P],
                                        vTs[:, j * P:(j + 1) * P],
                                    )

            if not USE_DMA_TRANSPOSE:
                with tc.tile_pool(name="ptr0", bufs=2, space="PSUM") as ptr0:
                    for j in range(JT):
                        tp_ = ptr0.tile([P, P], bf16, tag="tp0")
                        nc.tensor.transpose(
                            tp_[:], vTs[:, j * P:(j + 1) * P], ident[:]
                        )
                        nc.vector.tensor_copy(va[:, j, 0:P], tp_[:])

            # ---------------- phase 2: attention ----------------
            # PSUM: scores 2x[128,1024]f32 (4 banks) + avp [128,3,512]f32
            # (3 banks, 8 sub-bank regions at 132-f32 stride) = 7 banks.
            TOTJ = IB2 * G * JT  # 128 pipeline steps
            ets: dict = {}
            avp_of: dict = {}

            with tc.tile_pool(name="ps", bufs=2, space="PSUM") as ps, \
                 tc.tile_pool(name="pav", bufs=1, space="PSUM") as pav, \
                 (tc.tile_pool(name="ptr", bufs=1, space="PSUM")
                  if not USE_DMA_TRANSPOSE else _nullpool()) as ptr:

                def emit_av(idx):
                    blk, j = idx // JT, idx % JT
                    avp = avp_of[blk]
                    for isub in range(8):
                        b, r = isub // 3, isub % 3
                        c0 = r * RW
                        nc.tensor.matmul(
                            avp[:, b, c0:c0 + RW],
                            ets[idx][:, isub * P:(isub + 1) * P],
                            va[:, j, 0:RW],
                            start=(j == 0 and r == 0),
                            stop=(j == JT - 1 and (r == 2 or isub == 7)),
                        )

                def emit_norm(blk):
                    ib, g = blk // G, blk % G
                    avp = avp_of[blk]
                    tmp = tpool.tile([P, 3, 3 * RW], f32, tag="tmp")
                    for b in range(3):
                        # bank 2 holds only 2 regions; its spare slot is
                        # never matmul-written, so don't read it
                        w = 3 * RW if b < 2 else 2 * RW
                        nc.vector.tensor_copy(
                            tmp[:, b, 0:w], avp[:, b, 0:w]
                        )
                    for isub in range(8):
                        b, r = isub // 3, isub % 3
                        c0 = r * RW
                        rec = small.tile([P, 1], f32, tag="rec")
                        nc.vector.reciprocal(
                            rec[:], tmp[:, b, c0 + P:c0 + P + 1]
                        )
                        anorm = small.tile([P, P], bf16, name=f"an{isub}",
                                           tag=f"an{isub & 1}")
                        nc.vector.tensor_scalar_mul(
                            anorm[:], tmp[:, b, c0:c0 + P], rec[:]
                        )
                        col = ib * 1024 + isub * P
                        if USE_DMA_TRANSPOSE:
                            nc.sync.dma_start_transpose(
                                attnT[g][:, col:col + P], anorm[:]
                            )
                        else:
                            trp = ptr.tile([P, P], bf16, tag="trp")
                            nc.tensor.transpose(trp[:], anorm[:], ident[:])
                            nc.vector.tensor_copy(
                                attnT[g][:, col:col + P], trp[:]
                            )

                for idx in range(TOTJ + 1):
                    if idx < TOTJ:
                        blk, j = idx // JT, idx % JT
                        ib, g = blk // G, blk % G
                        if j == 0:
                            avp_of[blk] = pav.tile(
                                [P, 3, 512], f32, name=f"avp{blk}", tag="avp"
                            )
                        sps = ps.tile([P, 1024], f32, tag="sps")
                        for half in range(2):
                            nc.tensor.matmul(
                                sps[:, half * 512:(half + 1) * 512],
                                kT[:, j * P:(j + 1) * P],
                                qT[g][:, ib * 1024 + half * 512:
                                       ib * 1024 + (half + 1) * 512],
                                start=True,
                                stop=True,
                            )
                        et = etp.tile([P, 1024], bf16, tag="et")
                        nc.scalar.activation(
                            et[:], sps[:],
                            mybir.ActivationFunctionType.Exp,
                            scale=SCALE,
                        )
                        ets[idx] = et
                    if idx >= 1:
                        emit_av(idx - 1)
                        if (idx - 1) % JT == JT - 1:
                            emit_norm((idx - 1) // JT)

            # ---------------- phase 3: output projection ----------------
            # pass A uses only ib0 attnT columns, so it starts while the
            # last blocks' transposes are still in flight
            with tc.tile_pool(name="po", bufs=2, space="PSUM") as po:
                for half in range(2):
                    for eo in range(ET):
                        ops_ = [po.tile([P, 512], f32, name=f"op{k}",
                                        tag=f"op{k}") for k in range(2)]
                        for f in range(G):
                            for k in range(2):
                                nb = half * 2 + k
                                nc.tensor.matmul(
                                    ops_[k][:],
                                    wo_sb[:, f, eo * P:(eo + 1) * P],
                                    attnT[f][:, nb * 512:(nb + 1) * 512],
                                    start=(f == 0),
                                    stop=(f == G - 1),
                                )
                        ot = op.tile([P, 1024], bf16, tag=f"ot{half}")
                        for k in range(2):
                            nc.vector.tensor_copy(
                                ot[:, k * 512:(k + 1) * 512], ops_[k][:]
                            )
                            nc.sync.dma_start(
                                out=outT_d[eo, :,
                                           (half * 2 + k) * 512:
                                           (half * 2 + k + 1) * 512],
                                in_=ot[:, k * 512:(k + 1) * 512],
                            )
    nc.finalize()
    return nc


def _get_program():
    if "nc" not in _CACHE:
        _CACHE["nc"] = _build_program()
    return _CACHE["nc"]


def _make_in_maps(x, Wq, Wk, Wv, Wo):
    import ml_dtypes

    bf = ml_dtypes.bfloat16

    def wtile(w):  # [rows, E] -> [P, ET, rows] tiled on partition
        r = w.shape[0]
        return np.ascontiguousarray(
            w.T.reshape(ET, P, r).transpose(1, 0, 2)
        ).astype(bf)

    xT = [
        np.ascontiguousarray(x[b].T).astype(bf).reshape(ET, P, N) for b in range(B)
    ]
    in_maps = []
    for c in range(8):
        b, h = c // HKV, c % HKV
        wo = Wo[:, h * FQ:(h + 1) * FQ].T  # [FQ, E]
        in_maps.append({
            "xT": xT[b],
            "wqT": wtile(Wq[h * FQ:(h + 1) * FQ, :]),
            "wkT": wtile(Wk[h * D:(h + 1) * D, :]),
            "wvT": wtile(Wv[h * D:(h + 1) * D, :]),
            "woT": np.ascontiguousarray(
                wo.reshape(G, P, N).transpose(1, 0, 2)
            ).astype(bf),
        })
    return in_maps


def run_spmd(in_maps, trace=False, **kw):
    from concourse.bass_utils import run_bass_kernel_spmd

    nc = _get_program()
    return run_bass_kernel_spmd(nc, in_maps, list(range(8)), trace=trace, **kw)


def kernel(x, Wq, Wk, Wv, Wo, next_token_only=0, **_ignored):
    x = np.asarray(x, dtype=np.float32)
    Wq = np.asarray(Wq, dtype=np.float32)
    Wk = np.asarray(Wk, dtype=np.float32)
    Wv = np.asarray(Wv, dtype=np.float32)
    Wo = np.asarray(Wo, dtype=np.float32)

    res = run_spmd(_make_in_maps(x, Wq, Wk, Wv, Wo))
    outs = [np.asarray(r["outT"], dtype=np.float32).reshape(E, N)
            for r in res.results]
    full = np.empty((B, N, E), np.float32)
    for b in range(B):
        acc = outs[b * HKV].copy()
        for h in range(1, HKV):
            acc += outs[b * HKV + h]
        full[b] = acc.T
    return full
